# revision 26
# baseline (speedup 1.0000x reference)
"""Bass/Trainium2 kernel for nn_DynamicSparseTransformerBlock.

Sharding: 8 cores = batch(2) x spatial-quarter(4). Each core runs the five
1x1 convs (f,g,h,fp,gp) on its [256 x 4096] slab of pre-normalized inputs.
Gather/softmax/top-k index bookkeeping runs on host around the device convs.
"""

import numpy as np

B, H, W = 2, 128, 128
CQ, CP, CV, DQK, DP, K = 192, 64, 256, 256, 32, 8
EPS = 1e-10
NCORES = 8
NPC = 4096  # spatial columns per core (quarter of 128x128)

_CACHE = {}


def _build_nc():
    import concourse.bass as bass
    import concourse.mybir as mybir
    from concourse import bacc, tile

    dt = mybir.dt.float32
    nc = bacc.Bacc(None, target_bir_lowering=False)

    xq = nc.dram_tensor("xq", [256, NPC], dt, kind="ExternalInput")
    xk = nc.dram_tensor("xk", [256, NPC], dt, kind="ExternalInput")
    dtb = mybir.dt.bfloat16
    xv = nc.dram_tensor("xv", [256, NPC], dtb, kind="ExternalInput")
    # four fp32 weight matrices packed along the out dim: f|g|fp|gp
    wall = nc.dram_tensor("wall", [256, 576], dt, kind="ExternalInput")
    whb = nc.dram_tensor("whb", [256, 256], dtb, kind="ExternalInput")
    f_o = nc.dram_tensor("f", [256, NPC], dt, kind="ExternalOutput")
    g_o = nc.dram_tensor("g", [256, NPC], dt, kind="ExternalOutput")
    h_o = nc.dram_tensor("h", [256, NPC], dt, kind="ExternalOutput")
    fp_o = nc.dram_tensor("fp", [32, NPC], dt, kind="ExternalOutput")
    gp_o = nc.dram_tensor("gp", [32, NPC], dt, kind="ExternalOutput")

    NT = NPC // 512

    with tile.TileContext(nc) as tc:
        with (
            tc.tile_pool(name="wp", bufs=1) as wp,
            tc.tile_pool(name="xp", bufs=6) as xp,
            tc.tile_pool(name="ps", bufs=7, space="PSUM") as ps,
            tc.tile_pool(name="dps", bufs=1, space="PSUM") as dps,
            tc.tile_pool(name="op", bufs=3) as op,
            tc.tile_pool(name="dop", bufs=1) as dop,
        ):
            # Walrus allows only ONE sync wait on self-loading fp32 matmuls and
            # SWDGE DMAs. Trick: after each input DMA a 1-elem dummy matmul makes
            # PE observe that DMA's semaphore; after each output DMA a 1-elem
            # dummy DVE copy does the same for DVE. All real instructions then
            # carry at most one wait.
            dummy_ps = dps.tile([1, 1], dt, tag="dmy")

            def pe_fence(ap):
                nc.tensor.matmul(dummy_ps[:], ap, ap, start=True, stop=True)

            wt_all = wp.tile([128, 2, 576], dt, tag="wall")
            nc.sync.dma_start(wt_all[:], wall.rearrange("(a p) o -> p a o", p=128))
            pe_fence(wt_all[:, 0, :1])
            wt_h = wp.tile([128, 2, 256], dtb, tag="whb")
            nc.sync.dma_start(wt_h[:], whb.rearrange("(a p) o -> p a o", p=128))
            pe_fence(wt_h[:, 0, :1])
            w_off = {"wf": 0, "wg": 256, "wfp": 512, "wgp": 544}

            plans = [
                (xq, [("wf", f_o, 256), ("wfp", fp_o, 32)]),
                (xk, [("wg", g_o, 256), ("wgp", gp_o, 32)]),
                (xv, [("wh", h_o, 256)]),
            ]
            for xdram, jobs in plans:
                xv_r = xdram.rearrange("(a p) n -> p a n", p=128)
                xdt = dtb if xdram is xv else dt
                xt_c = []
                for cc in range(2):
                    xd = xp.tile([128, NPC], xdt, tag="x")
                    nc.sync.dma_start(xd[:], xv_r[:, cc, :])
                    pe_fence(xd[:, :1])
                    xt_c.append(xd)
                for wname, odram, co in jobs:
                    woff = w_off.get(wname, 0)
                    ovw = odram.rearrange("(a p) n -> p a n", p=128) if co == 256 else None
                    for oc in range(co // 128 if co >= 128 else 1):
                        ow = 128 if co == 256 else 32
                        stage = op.tile([ow, NPC], dt, tag="st")
                        for nt in range(NT):
                            pt = ps.tile([ow, 512], dt, tag="ps")
                            for cc in range(2):
                                rhs = xt_c[cc][:, nt * 512 : (nt + 1) * 512]
                                if wname == "wh":
                                    wsrc = wt_h[:, cc, oc * 128 : oc * 128 + ow]
                                else:
                                    wsrc = wt_all[
                                        :, cc, woff + oc * 128 : woff + oc * 128 + ow
                                    ]
                                nc.tensor.matmul(
                                    pt[:],
                                    wsrc,
                                    rhs,
                                    start=(cc == 0),
                                    stop=(cc == 1),
                                )
                            # alternate PSUM->SBUF copies across DVE and ACT
                            # so the copy stream isn't serialized on one engine
                            if nt % 2 == 0:
                                nc.vector.tensor_copy(
                                    stage[:, nt * 512 : (nt + 1) * 512], pt[:]
                                )
                            else:
                                nc.scalar.copy(
                                    stage[:, nt * 512 : (nt + 1) * 512], pt[:]
                                )
                        if co == 256:
                            nc.sync.dma_start(ovw[:, oc, :], stage[:])
                        else:
                            nc.sync.dma_start(odram[:, :], stage[:])
                        # DVE fence on the out-DMA so stage recycling never
                        # stacks a DMA wait onto a later TensorCopy
                        nc.vector.tensor_copy(stage[:1, :1], stage[:1, 1:2])
    nc.compile()
    return nc


def _run_device(query, key, value, WfT, WgT, WhT, WfpT, WgpT, trace=False):
    """query/key/value: [B, 256, H, W] fp32. Returns f,g,h [B,256,H,W], fp,gp [B,32,H,W]."""
    from concourse import bass_utils

    if "nc" not in _CACHE:
        _CACHE["nc"] = _build_nc()
    nc = _CACHE["nc"]

    import ml_dtypes
    wall = np.concatenate([WfT, WgT, WfpT, WgpT], axis=1)
    wall = np.ascontiguousarray(wall, np.float32)  # [256, 576]
    whb = np.ascontiguousarray(WhT.astype(ml_dtypes.bfloat16))
    in_maps = []
    for core in range(NCORES):
        bi, qt = core // 4, core % 4
        sl = np.s_[bi, :, 32 * qt : 32 * qt + 32, :]
        in_maps.append(
            {
                "xq": np.ascontiguousarray(query[sl].reshape(256, NPC)),
                "xk": np.ascontiguousarray(key[sl].reshape(256, NPC)),
                "xv": np.ascontiguousarray(value[sl].reshape(256, NPC).astype(ml_dtypes.bfloat16)),
                "wall": wall, "whb": whb,
            }
        )

    import time as _time
    try:
        t0 = _time.time()
        br = bass_utils.run_bass_kernel_spmd(
            nc, in_maps, core_ids=list(range(NCORES)), trace=trace
        )
    except ModuleNotFoundError:
        t0 = _time.time()
        br = bass_utils.run_bass_kernel_spmd(
            nc, in_maps, core_ids=list(range(NCORES)), trace=False
        )
    _CACHE["device_wall_ns"] = int((_time.time() - t0) * 1e9)
    _CACHE["last_exec_time_ns"] = br.exec_time_ns

    f = np.empty((B, 256, H, W), np.float32)
    g = np.empty((B, 256, H, W), np.float32)
    h = np.empty((B, 256, H, W), np.float32)
    fp = np.empty((B, 32, H, W), np.float32)
    gp = np.empty((B, 32, H, W), np.float32)
    for core in range(NCORES):
        bi, qt = core // 4, core % 4
        r = br.results[core]
        sl = np.s_[bi, :, 32 * qt : 32 * qt + 32, :]
        f[sl] = r["f"].reshape(256, 32, W)
        g[sl] = r["g"].reshape(256, 32, W)
        h[sl] = r["h"].reshape(256, 32, W)
        fp[sl] = r["fp"].reshape(32, 32, W)
        gp[sl] = r["gp"].reshape(32, 32, W)
    return f, g, h, fp, gp


def _unfold2(x):
    b, c, Hh, Ww = x.shape
    x = x.reshape(b, c, Hh // 2, 2, Ww // 2, 2)
    x = x.transpose(0, 2, 4, 3, 5, 1)
    return np.ascontiguousarray(x.reshape(b, (Hh // 2) * (Ww // 2), 4, c))


def _fold2(y, Hh, Ww):
    b, L, _, c = y.shape
    y = y.reshape(b, Hh // 2, Ww // 2, 2, 2, c)
    y = y.transpose(0, 5, 1, 3, 2, 4)
    return np.ascontiguousarray(y.reshape(b, c, Hh, Ww))


def kernel(q, k, v, pos, seg_map, prev_attn_top_k_idx, Wf, bf, Wg, bg, Wh, bh,
           Wfp, bfp, Wgp, bgp, _trace=False):
    q = np.asarray(q, np.float32)
    k = np.asarray(k, np.float32)
    v = np.asarray(v, np.float32)
    pos = np.asarray(pos, np.float32)
    idx = np.asarray(prev_attn_top_k_idx)

    # feature-normalize over channels + concat pos (host; ~0.1% of FLOPs)
    def fnorm(x):
        n = np.sqrt(np.sum(x * x, axis=1, keepdims=True, dtype=np.float32)).astype(np.float32) + np.float32(EPS)
        return (x / n).astype(np.float32)

    pos_b = np.broadcast_to(pos, (B,) + pos.shape[1:])
    query = np.concatenate([fnorm(q), pos_b], axis=1)
    key = np.concatenate([fnorm(k), pos_b], axis=1)

    # device: the five 1x1 convs (weights pre-transposed to [c_in, c_out])
    f, g, h, fp, gp = _run_device(
        query, key, v,
        np.ascontiguousarray(Wf.T.astype(np.float32)),
        np.ascontiguousarray(Wg.T.astype(np.float32)),
        np.ascontiguousarray(Wh.T.astype(np.float32)),
        np.ascontiguousarray(Wfp.T.astype(np.float32)),
        np.ascontiguousarray(Wgp.T.astype(np.float32)),
        trace=_trace,
    )
    f += bf[None, :, None, None].astype(np.float32)
    g += bg[None, :, None, None].astype(np.float32)
    h += bh[None, :, None, None].astype(np.float32)
    fp += bfp[None, :, None, None].astype(np.float32)
    gp += bgp[None, :, None, None].astype(np.float32)

    smooth = np.float32(np.sqrt(Wf.shape[0]))

    # ---- inter_scale_attn (host glue; index bookkeeping + small einsums) ----
    qw = _unfold2(f)      # [B,Nq,4,256]
    qpw = _unfold2(fp)
    kw = _unfold2(g)      # [B,Nk,4,256]
    kpw = _unfold2(gp)
    vw = _unfold2(h)
    b, Nq = qw.shape[0], qw.shape[1]
    Hk = Wk = 128

    out = np.empty((B, Nq, 4, CV), np.float32)
    conf = np.empty((B, Nq, 4, 1), np.float32)
    new_idx = np.empty((B, H * W, K), np.int32)

    for bi in range(B):
        qk = kw[bi][idx[bi]].reshape(Nq, K * 4, DQK)      # [Nq,32,256]
        qkp = kpw[bi][idx[bi]].reshape(Nq, K * 4, DP)
        qv = vw[bi][idx[bi]].reshape(Nq, K * 4, CV)
        scores = np.matmul(qw[bi], qk.transpose(0, 2, 1)) / smooth  # [Nq,4,32]
        m = scores.max(axis=-1, keepdims=True)
        e = np.exp(scores - m, dtype=np.float32)
        attn = e / e.sum(axis=-1, keepdims=True, dtype=np.float32)
        maskd = np.matmul(qpw[bi], qkp.transpose(0, 2, 1))
        mask = (maskd > 0).astype(np.float32)
        masked = mask * attn
        out[bi] = np.matmul(masked, qv)
        conf[bi] = masked.sum(-1)[..., None]

        # re-select top-k -> full-res key grid
        W2k = Wk // 2
        iy = idx[bi] // W2k
        ix = idx[bi] % W2k
        base = 2 * iy * Wk + 2 * ix
        cand = np.stack([base, base + 1, base + Wk, base + Wk + 1], axis=-1)
        cand = cand.reshape(Nq, K * 4)
        tki = np.argsort(-attn, axis=-1, kind="stable")[..., :K]   # [Nq,4,K]
        cand_b = np.broadcast_to(cand[:, None, :], (Nq, 4, K * 4))
        ni = np.take_along_axis(cand_b, tki, axis=-1)              # [Nq,4,K]
        ni = ni.reshape(H // 2, W // 2, 2, 2, K)
        ni = ni.transpose(0, 2, 1, 3, 4).reshape(H * W, K)
        new_idx[bi] = ni.astype(np.int32)

    out_f = _fold2(out, H, W)
    conf_f = _fold2(conf, H, W)
    return out_f, new_idx, conf_f


# revision 27
# speedup vs baseline: 1.1394x; 1.1394x over previous
"""Bass/Trainium2 kernel for nn_DynamicSparseTransformerBlock.

Sharding: 8 cores = batch(2) x spatial-quarter(4). Each core runs the five
1x1 convs (f,g,h,fp,gp) on its [256 x 4096] slab of pre-normalized inputs.
Gather/softmax/top-k index bookkeeping runs on host around the device convs.
"""

import numpy as np

B, H, W = 2, 128, 128
CQ, CP, CV, DQK, DP, K = 192, 64, 256, 256, 32, 8
EPS = 1e-10
NCORES = 8
NPC = 4096  # spatial columns per core (quarter of 128x128)

_CACHE = {}


def _build_nc():
    import concourse.bass as bass
    import concourse.mybir as mybir
    from concourse import bacc, tile

    dt = mybir.dt.float32
    nc = bacc.Bacc(None, target_bir_lowering=False)

    xq = nc.dram_tensor("xq", [256, NPC], dt, kind="ExternalInput")
    xk = nc.dram_tensor("xk", [256, NPC], dt, kind="ExternalInput")
    dtb = mybir.dt.bfloat16
    xv = nc.dram_tensor("xv", [256, NPC], dtb, kind="ExternalInput")
    # four fp32 weight matrices packed along the out dim: f|g|fp|gp
    wall = nc.dram_tensor("wall", [256, 576], dt, kind="ExternalInput")
    whb = nc.dram_tensor("whb", [256, 256], dtb, kind="ExternalInput")
    f_o = nc.dram_tensor("f", [256, NPC], dt, kind="ExternalOutput")
    g_o = nc.dram_tensor("g", [256, NPC], dt, kind="ExternalOutput")
    h_o = nc.dram_tensor("h", [256, NPC], dtb, kind="ExternalOutput")
    fp_o = nc.dram_tensor("fp", [32, NPC], dt, kind="ExternalOutput")
    gp_o = nc.dram_tensor("gp", [32, NPC], dt, kind="ExternalOutput")

    NT = NPC // 512

    with tile.TileContext(nc) as tc:
        with (
            tc.tile_pool(name="wp", bufs=1) as wp,
            tc.tile_pool(name="xp", bufs=6) as xp,
            tc.tile_pool(name="ps", bufs=7, space="PSUM") as ps,
            tc.tile_pool(name="dps", bufs=1, space="PSUM") as dps,
            tc.tile_pool(name="op", bufs=3) as op,
            tc.tile_pool(name="dop", bufs=1) as dop,
        ):
            # Walrus allows only ONE sync wait on self-loading fp32 matmuls and
            # SWDGE DMAs. Trick: after each input DMA a 1-elem dummy matmul makes
            # PE observe that DMA's semaphore; after each output DMA a 1-elem
            # dummy DVE copy does the same for DVE. All real instructions then
            # carry at most one wait.
            dummy_ps = dps.tile([1, 1], dt, tag="dmy")

            def pe_fence(ap):
                nc.tensor.matmul(dummy_ps[:], ap, ap, start=True, stop=True)

            wt_all = wp.tile([128, 2, 576], dt, tag="wall")
            nc.sync.dma_start(wt_all[:], wall.rearrange("(a p) o -> p a o", p=128))
            pe_fence(wt_all[:, 0, :1])
            wt_h = wp.tile([128, 2, 256], dtb, tag="whb")
            nc.sync.dma_start(wt_h[:], whb.rearrange("(a p) o -> p a o", p=128))
            pe_fence(wt_h[:, 0, :1])
            w_off = {"wf": 0, "wg": 256, "wfp": 512, "wgp": 544}

            plans = [
                (xq, [("wf", f_o, 256), ("wfp", fp_o, 32)]),
                (xk, [("wg", g_o, 256), ("wgp", gp_o, 32)]),
                (xv, [("wh", h_o, 256)]),
            ]
            for xdram, jobs in plans:
                xv_r = xdram.rearrange("(a p) n -> p a n", p=128)
                xdt = dtb if xdram is xv else dt
                xt_c = []
                for cc in range(2):
                    xd = xp.tile([128, NPC], xdt, tag="x")
                    nc.sync.dma_start(xd[:], xv_r[:, cc, :])
                    pe_fence(xd[:, :1])
                    xt_c.append(xd)
                for wname, odram, co in jobs:
                    woff = w_off.get(wname, 0)
                    ovw = odram.rearrange("(a p) n -> p a n", p=128) if co == 256 else None
                    for oc in range(co // 128 if co >= 128 else 1):
                        ow = 128 if co == 256 else 32
                        sdt = dtb if wname == "wh" else dt
                        stage = op.tile([ow, NPC], sdt, tag="st")
                        for nt in range(NT):
                            pt = ps.tile([ow, 512], dt, tag="ps")
                            for cc in range(2):
                                rhs = xt_c[cc][:, nt * 512 : (nt + 1) * 512]
                                if wname == "wh":
                                    wsrc = wt_h[:, cc, oc * 128 : oc * 128 + ow]
                                else:
                                    wsrc = wt_all[
                                        :, cc, woff + oc * 128 : woff + oc * 128 + ow
                                    ]
                                nc.tensor.matmul(
                                    pt[:],
                                    wsrc,
                                    rhs,
                                    start=(cc == 0),
                                    stop=(cc == 1),
                                )
                            # alternate PSUM->SBUF copies across DVE and ACT
                            # so the copy stream isn't serialized on one engine
                            if nt % 2 == 0:
                                nc.vector.tensor_copy(
                                    stage[:, nt * 512 : (nt + 1) * 512], pt[:]
                                )
                            else:
                                nc.scalar.copy(
                                    stage[:, nt * 512 : (nt + 1) * 512], pt[:]
                                )
                        if co == 256:
                            nc.sync.dma_start(ovw[:, oc, :], stage[:])
                        else:
                            nc.sync.dma_start(odram[:, :], stage[:])
                        # DVE fence on the out-DMA so stage recycling never
                        # stacks a DMA wait onto a later TensorCopy
                        nc.vector.tensor_copy(stage[:1, :1], stage[:1, 1:2])
    nc.compile()
    return nc


def _run_device(query, key, value, WfT, WgT, WhT, WfpT, WgpT, trace=False):
    """query/key/value: [B, 256, H, W] fp32. Returns f,g,h [B,256,H,W], fp,gp [B,32,H,W]."""
    from concourse import bass_utils

    if "nc" not in _CACHE:
        _CACHE["nc"] = _build_nc()
    nc = _CACHE["nc"]

    import ml_dtypes
    wall = np.concatenate([WfT, WgT, WfpT, WgpT], axis=1)
    wall = np.ascontiguousarray(wall, np.float32)  # [256, 576]
    whb = np.ascontiguousarray(WhT.astype(ml_dtypes.bfloat16))
    in_maps = []
    for core in range(NCORES):
        bi, qt = core // 4, core % 4
        sl = np.s_[bi, :, 32 * qt : 32 * qt + 32, :]
        in_maps.append(
            {
                "xq": np.ascontiguousarray(query[sl].reshape(256, NPC)),
                "xk": np.ascontiguousarray(key[sl].reshape(256, NPC)),
                "xv": np.ascontiguousarray(value[sl].reshape(256, NPC).astype(ml_dtypes.bfloat16)),
                "wall": wall, "whb": whb,
            }
        )

    import time as _time
    try:
        t0 = _time.time()
        br = bass_utils.run_bass_kernel_spmd(
            nc, in_maps, core_ids=list(range(NCORES)), trace=trace
        )
    except ModuleNotFoundError:
        t0 = _time.time()
        br = bass_utils.run_bass_kernel_spmd(
            nc, in_maps, core_ids=list(range(NCORES)), trace=False
        )
    _CACHE["device_wall_ns"] = int((_time.time() - t0) * 1e9)
    _CACHE["last_exec_time_ns"] = br.exec_time_ns

    f = np.empty((B, 256, H, W), np.float32)
    g = np.empty((B, 256, H, W), np.float32)
    h = np.empty((B, 256, H, W), np.float32)
    fp = np.empty((B, 32, H, W), np.float32)
    gp = np.empty((B, 32, H, W), np.float32)
    for core in range(NCORES):
        bi, qt = core // 4, core % 4
        r = br.results[core]
        sl = np.s_[bi, :, 32 * qt : 32 * qt + 32, :]
        f[sl] = r["f"].reshape(256, 32, W)
        g[sl] = r["g"].reshape(256, 32, W)
        h[sl] = np.asarray(r["h"], np.float32).reshape(256, 32, W)
        fp[sl] = r["fp"].reshape(32, 32, W)
        gp[sl] = r["gp"].reshape(32, 32, W)
    return f, g, h, fp, gp


def _unfold2(x):
    b, c, Hh, Ww = x.shape
    x = x.reshape(b, c, Hh // 2, 2, Ww // 2, 2)
    x = x.transpose(0, 2, 4, 3, 5, 1)
    return np.ascontiguousarray(x.reshape(b, (Hh // 2) * (Ww // 2), 4, c))


def _fold2(y, Hh, Ww):
    b, L, _, c = y.shape
    y = y.reshape(b, Hh // 2, Ww // 2, 2, 2, c)
    y = y.transpose(0, 5, 1, 3, 2, 4)
    return np.ascontiguousarray(y.reshape(b, c, Hh, Ww))


def kernel(q, k, v, pos, seg_map, prev_attn_top_k_idx, Wf, bf, Wg, bg, Wh, bh,
           Wfp, bfp, Wgp, bgp, _trace=False):
    q = np.asarray(q, np.float32)
    k = np.asarray(k, np.float32)
    v = np.asarray(v, np.float32)
    pos = np.asarray(pos, np.float32)
    idx = np.asarray(prev_attn_top_k_idx)

    # feature-normalize over channels + concat pos (host; ~0.1% of FLOPs)
    def fnorm(x):
        n = np.sqrt(np.sum(x * x, axis=1, keepdims=True, dtype=np.float32)).astype(np.float32) + np.float32(EPS)
        return (x / n).astype(np.float32)

    pos_b = np.broadcast_to(pos, (B,) + pos.shape[1:])
    query = np.concatenate([fnorm(q), pos_b], axis=1)
    key = np.concatenate([fnorm(k), pos_b], axis=1)

    # device: the five 1x1 convs (weights pre-transposed to [c_in, c_out])
    f, g, h, fp, gp = _run_device(
        query, key, v,
        np.ascontiguousarray(Wf.T.astype(np.float32)),
        np.ascontiguousarray(Wg.T.astype(np.float32)),
        np.ascontiguousarray(Wh.T.astype(np.float32)),
        np.ascontiguousarray(Wfp.T.astype(np.float32)),
        np.ascontiguousarray(Wgp.T.astype(np.float32)),
        trace=_trace,
    )
    f += bf[None, :, None, None].astype(np.float32)
    g += bg[None, :, None, None].astype(np.float32)
    h += bh[None, :, None, None].astype(np.float32)
    fp += bfp[None, :, None, None].astype(np.float32)
    gp += bgp[None, :, None, None].astype(np.float32)

    smooth = np.float32(np.sqrt(Wf.shape[0]))

    # ---- inter_scale_attn (host glue; index bookkeeping + small einsums) ----
    qw = _unfold2(f)      # [B,Nq,4,256]
    qpw = _unfold2(fp)
    kw = _unfold2(g)      # [B,Nk,4,256]
    kpw = _unfold2(gp)
    vw = _unfold2(h)
    b, Nq = qw.shape[0], qw.shape[1]
    Hk = Wk = 128

    out = np.empty((B, Nq, 4, CV), np.float32)
    conf = np.empty((B, Nq, 4, 1), np.float32)
    new_idx = np.empty((B, H * W, K), np.int32)

    for bi in range(B):
        qk = kw[bi][idx[bi]].reshape(Nq, K * 4, DQK)      # [Nq,32,256]
        qkp = kpw[bi][idx[bi]].reshape(Nq, K * 4, DP)
        qv = vw[bi][idx[bi]].reshape(Nq, K * 4, CV)
        scores = np.matmul(qw[bi], qk.transpose(0, 2, 1)) / smooth  # [Nq,4,32]
        m = scores.max(axis=-1, keepdims=True)
        e = np.exp(scores - m, dtype=np.float32)
        attn = e / e.sum(axis=-1, keepdims=True, dtype=np.float32)
        maskd = np.matmul(qpw[bi], qkp.transpose(0, 2, 1))
        mask = (maskd > 0).astype(np.float32)
        masked = mask * attn
        out[bi] = np.matmul(masked, qv)
        conf[bi] = masked.sum(-1)[..., None]

        # re-select top-k -> full-res key grid
        W2k = Wk // 2
        iy = idx[bi] // W2k
        ix = idx[bi] % W2k
        base = 2 * iy * Wk + 2 * ix
        cand = np.stack([base, base + 1, base + Wk, base + Wk + 1], axis=-1)
        cand = cand.reshape(Nq, K * 4)
        tki = np.argsort(-attn, axis=-1, kind="stable")[..., :K]   # [Nq,4,K]
        cand_b = np.broadcast_to(cand[:, None, :], (Nq, 4, K * 4))
        ni = np.take_along_axis(cand_b, tki, axis=-1)              # [Nq,4,K]
        ni = ni.reshape(H // 2, W // 2, 2, 2, K)
        ni = ni.transpose(0, 2, 1, 3, 4).reshape(H * W, K)
        new_idx[bi] = ni.astype(np.int32)

    out_f = _fold2(out, H, W)
    conf_f = _fold2(conf, H, W)
    return out_f, new_idx, conf_f
